# revision 16
# baseline (speedup 1.0000x reference)
"""Trainium2 Bass kernel for BaselineFeedforwardNetwork (dense_mlp).

Computation (per path n, step t):
    x_t   = [f_t (3), delta_{t-1} (1)]
    h     = relu(x_t @ W1 + b1)        # 4  -> 64
    h2    = relu(h @ W2 + b2)          # 64 -> 64
    delta = h2 @ W3 + b3               # 64 -> 1
Output: deltas (N, T).

Strategy (8 NeuronCores, pure data parallel over N):
  * 32768 paths/core; "pair" = 1024 paths = 2 chunks of 512 stacked on
    the 128 partitions (chunk A -> hidden rows 0:64, chunk B -> 64:128)
  * all matmuls are full-width K=128 (or K<=8 for layer 1) with
    block-diagonal weights -> 3 matmuls x 512 columns per pair-step,
    the minimum PE streaming for this net (PE issue is serial on TRN2)
  * delta feedback is direct: layer-3 deltas for 8 pairs accumulate
    into one [16,512] PSUM tile via column-select lhsT, ACT copies
    them to SBUF (bf16, +b3), small SBUF->SBUF DMAs scatter them into
    the delta rows of the NEXT step's x tile; block ordering hides the
    recurrence latency behind the other blocks' compute
  * x tile per step: [128, 4096] bf16; partition group 32g holds rows
    [fA(3), fB(3), dA, dB] for pairs p=4q+g at free cols 512q
  * features pre-packed on host to [T, 4, 6, 4096] so each step's
    feature load is one large contiguous DMA
"""

import sys

for _p in ("/opt/trn_rl_repo",):
    if _p not in sys.path:
        sys.path.insert(0, _p)

import numpy as np
import ml_dtypes

NCORES = 8
N_TOT, T, FDIM = 262144, 60, 3
NC = N_TOT // NCORES          # 32768 paths per core
HID = 64
CH = 512                      # matmul free dim (one PSUM bank of fp32)
NPAIR = 32                    # pairs (1024 paths) per core-step
NBLK = 2                      # layer-3 blocks of 16 pairs
XPRE = 4                      # steps of feature prefetch

# wpack column layout (bf16, 128 partitions)
W1_OFF = 0                    # cols 0:128, rows 32g+{0..7}: L1 lhsT per group
W2_OFF = 128                  # cols 128:256: blockdiag(W2, W2)
W3_OFF = 256                  # cols 256:768: 16 x [128,32] L3 select lhsT
WCOLS = 768


def _build_graph(nsteps=T, b3val=0.0):
    import concourse.bacc as bacc
    from concourse import mybir
    from concourse.tile import TileContext

    BF = mybir.dt.bfloat16
    F32 = mybir.dt.float32

    import time as _time

    nc = bacc.Bacc(trn_type="TRN2", name=f"k{int(_time.time())}")

    feats_p = nc.declare_dram_parameter("feats", [nsteps, 4, 6, 8 * CH], BF,
                                        isOutput=False)
    wpack_p = nc.declare_dram_parameter("wpack", [128, WCOLS], BF,
                                        isOutput=False)
    bias_p = nc.declare_dram_parameter("biasp", [128, 2], F32, isOutput=False)
    out_p = nc.declare_dram_parameter("out", [nsteps, 32, NBLK * CH], BF,
                                      isOutput=True)

    with TileContext(nc) as tc:
        with (
            tc.tile_pool(name="consts", bufs=1) as cpool,
            tc.tile_pool(name="xq", bufs=XPRE + 2) as xpool,
            tc.tile_pool(name="hh", bufs=3) as hpool,
            tc.tile_pool(name="dst", bufs=2) as dpool,
            tc.tile_pool(name="ps", bufs=3, space="PSUM") as ppool,
            tc.tile_pool(name="ps3", bufs=2, space="PSUM") as ppool3,
        ):
            wp = cpool.tile([128, WCOLS], BF, tag="wpack")
            bp = cpool.tile([128, 2], F32, tag="biasp")
            nc.sync.dma_start(out=wp[:, :], in_=wpack_p[:, :])
            nc.sync.dma_start(out=bp[:, :], in_=bias_p[:, :])

            # Warm-up: loads the ACT table + lets ACT/DVE observe const DMAs
            warm = cpool.tile([128, 4], F32, tag="warm")
            nc.scalar.activation(
                warm[:, 0:1], bp[:, 0:1],
                mybir.ActivationFunctionType.Relu, bias=0.0, scale=1.0,
            )
            nc.vector.tensor_scalar(
                warm[:, 1:2], bp[:, 1:2], 0.0, None, mybir.AluOpType.add,
            )

            # PE warm-up burst: ~8 us of dense back-to-back matmuls trips
            # the HAM activity monitor to K=8/8 (2.4 GHz) before the main
            # loop; overlaps the constant/feature DMAs.  Without it the
            # clock stays at the cold 1.2 GHz default for the whole kernel
            # (HAM is free-running; kernel start state is luck otherwise).
            dum = cpool.tile([128, 640], BF, tag="dum")
            nc.gpsimd.memset(dum[:, :], 0.0)
            Pw = ppool.tile([128, 2 * CH], F32, tag="pre")
            for _ in range(24):
                nc.tensor.matmul(
                    Pw[:, 0:CH], dum[:, 0:128], dum[:, 128:640],
                    start=True, stop=True,
                )

            def dma_x(t0):
                xt = xpool.tile([128, 8 * CH], BF, tag="x")
                for g in range(4):
                    nc.sync.dma_start(
                        out=xt[32 * g: 32 * g + 6, :], in_=feats_p[t0, g]
                    )
                return xt

            xq = [dma_x(t0) for t0 in range(min(XPRE, nsteps))]

            # Software-pipelined duo stream: at position D we emit L1 of
            # duo D, L2 of duo D-1, L3 of duo D-2 — every matmul's
            # cross-engine input (ACT's h, DVE's h2) was produced a full
            # duo (~1.5 us) earlier, so the PE never waits.
            # duo local index d: block b = d//8, qd = 2b + (d%8)//4,
            # g = d%4.
            NDUO = 16
            total = nsteps * NDUO
            duos = {}
            acc3s = {}
            dstages = {}

            def emit_l1(D):
                t, d = divmod(D, NDUO)
                if d == 0 and t + XPRE < nsteps:
                    xq.append(dma_x(t + XPRE))
                b, g = d // 8, d % 4
                qd = 2 * b + (d % 8) // 4
                xt = xq[t]
                P = ppool.tile([128, 2 * CH], F32, tag="pre")
                h = hpool.tile([128, 2 * CH], BF, tag="h")
                K1 = 6 if t == 0 else 8
                for qq in range(2):
                    q = 2 * qd + qq
                    nc.tensor.matmul(
                        P[:, CH * qq: CH * (qq + 1)],
                        wp[32 * g: 32 * g + K1, W1_OFF: W1_OFF + 128],
                        xt[32 * g: 32 * g + K1, CH * q: CH * (q + 1)],
                        start=True, stop=True,
                        tile_position=(32 * g, 0),
                    )
                nc.scalar.activation(
                    h[:, :], P[:, :],
                    mybir.ActivationFunctionType.Relu,
                    bias=bp[:, 0:1], scale=1.0,
                )
                duos[D] = (P, h)

            def emit_l2(D):
                P, h = duos[D]
                h2 = hpool.tile([128, 2 * CH], BF, tag="h2")
                for qq in range(2):
                    nc.tensor.matmul(
                        P[:, CH * qq: CH * (qq + 1)],
                        wp[:, W2_OFF: W2_OFF + 128],
                        h[:, CH * qq: CH * (qq + 1)],
                        start=True, stop=True,
                    )
                nc.vector.tensor_scalar(
                    h2[:, :], P[:, :],
                    bp[:, 1:2], 0.0,
                    mybir.AluOpType.add, mybir.AluOpType.max,
                )
                duos[D] = (None, h2)

            def emit_l3(D):
                t, d = divmod(D, NDUO)
                b, g = d // 8, d % 4
                qd = 2 * b + (d % 8) // 4
                _, h2 = duos.pop(D)
                if d % 8 == 0:
                    acc3s[t, b] = ppool3.tile([32, CH], F32, tag="l3acc",
                                           name=f"a3_{t}_{b}")
                acc3 = acc3s[t, b]
                i3 = 2 * (d % 8)
                for qq in range(2):
                    qloc = 2 * (qd - 2 * b) + qq
                    k = 4 * qloc + g
                    nc.tensor.matmul(
                        acc3[0:32, :],
                        wp[:, W3_OFF + 32 * k: W3_OFF + 32 * (k + 1)],
                        h2[:, CH * qq: CH * (qq + 1)],
                        start=(i3 + qq == 0), stop=(i3 + qq == 15),
                    )
                if d % 8 == 7:
                    # block complete: act3 + scatter (+ step out DMA)
                    if t not in dstages:
                        dstages[t] = dpool.tile([32, NBLK * CH], BF,
                                                tag="dstage",
                                                name=f"ds_{t}")
                    dstage = dstages[t]
                    nc.scalar.activation(
                        dstage[0:32, CH * b: CH * (b + 1)],
                        acc3s.pop((t, b))[0:32, :],
                        mybir.ActivationFunctionType.Copy,
                        bias=float(b3val), scale=1.0,
                    )
                    if t + 1 < nsteps:
                        xnext = xq[t + 1]
                        for c in range(2):
                            nc.sync.dma_start(
                                out=xnext.rearrange(
                                    "(g r) (q4 m) -> g r q4 m", g=4, q4=8
                                )[:, 6 + c, 4 * b: 4 * b + 4, :],
                                in_=dstage[16 * c: 16 * c + 16,
                                           CH * b: CH * (b + 1)],
                            )
                    if b == NBLK - 1:
                        nc.sync.dma_start(out=out_p[t],
                                          in_=dstages.pop(t)[0:32, :])

            for D in range(total + 2):
                if D < total:
                    emit_l1(D)
                if 1 <= D <= total:
                    emit_l2(D - 1)
                if 2 <= D <= total + 1:
                    emit_l3(D - 2)
    return nc


LAST_RESULT = None


def kernel(**inputs):
    return _run(inputs, T)


def _prepare(inputs, nsteps):
    features = np.asarray(inputs["features"], dtype=np.float32)
    W1 = np.asarray(inputs["W1"], dtype=np.float32)
    b1 = np.asarray(inputs["b1"], dtype=np.float32)
    W2 = np.asarray(inputs["W2"], dtype=np.float32)
    b2 = np.asarray(inputs["b2"], dtype=np.float32)
    W3 = np.asarray(inputs["W3"], dtype=np.float32)
    b3 = np.asarray(inputs["b3"], dtype=np.float32)

    nc = _build_graph(nsteps, float(b3[0]))
    nc.finalize()

    bf = ml_dtypes.bfloat16

    # wpack
    wpack = np.zeros((128, WCOLS), np.float32)
    for g in range(4):
        for c in range(2):
            for f in range(FDIM):
                wpack[32 * g + 3 * c + f,
                      W1_OFF + 64 * c: W1_OFF + 64 * (c + 1)] = W1[f]
            wpack[32 * g + 6 + c,
                  W1_OFF + 64 * c: W1_OFF + 64 * (c + 1)] = W1[3]
    for c in range(2):
        wpack[64 * c: 64 * (c + 1),
              W2_OFF + 64 * c: W2_OFF + 64 * (c + 1)] = W2
    for k in range(16):  # k = 4*qloc + g; acc3 rows m = 16c + 4g + qloc
        qloc, g = divmod(k, 4)
        for c in range(2):
            m = 16 * c + 4 * g + qloc
            wpack[64 * c: 64 * (c + 1), W3_OFF + 32 * k + m] = W3[:, 0]
    wpack = wpack.astype(bf)

    biasp = np.zeros((128, 2), np.float32)
    for half in (0, 64):
        biasp[half: half + HID, 0] = b1
        biasp[half: half + HID, 1] = b2

    # features: path = 4096 q + 1024 g + 512 c + j
    # host layout F[t, g, 3c+f, 512 q + j] = features[path, t, f]
    in_maps = []
    for k in range(NCORES):
        sh = features[k * NC: (k + 1) * NC, :nsteps, :]   # (NC, ns, 3)
        v = sh.reshape(8, 4, 2, CH, nsteps, FDIM)          # q g c j t f
        v = v.transpose(4, 1, 2, 5, 0, 3)                  # t g c f q j
        feats = np.ascontiguousarray(v).reshape(
            nsteps, 4, 6, 8 * CH).astype(bf)
        in_maps.append({"feats": feats, "wpack": wpack, "biasp": biasp})

    return nc, in_maps


def _unscramble(o, nsteps):
    # o: (ns, 32, 1024) bf16; rows m = 16 c + 4 g + qloc, cols = 512 b + j
    # path = 4096 (4b + qloc) + 1024 g + 512 c + j
    v = np.asarray(o).astype(np.float32)
    v = v.reshape(nsteps, 2, 4, 4, NBLK, CH)        # t c g qloc b j
    v = v.transpose(4, 3, 2, 1, 5, 0)               # b qloc g c j t
    return v.reshape(NC, nsteps)


def _run(inputs, nsteps, trace=False):
    global LAST_RESULT
    from concourse.bass_utils import run_bass_kernel_spmd

    nc, in_maps = _prepare(inputs, nsteps)
    res = run_bass_kernel_spmd(
        nc, in_maps, core_ids=list(range(NCORES)), trace=trace
    )
    LAST_RESULT = res
    outs = res.results

    full = np.empty((N_TOT, nsteps), np.float32)
    for k in range(NCORES):
        full[k * NC: (k + 1) * NC] = _unscramble(outs[k]["out"], nsteps)
    return full


if __name__ == "__main__":
    import reference

    inputs = reference.setup_inputs()
    out = kernel(**{k: np.asarray(v) for k, v in inputs.items()})
    print("kernel out", out.shape, out.dtype)


# revision 17
# speedup vs baseline: 2.0035x; 2.0035x over previous
"""Trainium2 Bass kernel for BaselineFeedforwardNetwork (dense_mlp).

Computation (per path n, step t):
    x_t   = [f_t (3), delta_{t-1} (1)]
    h     = relu(x_t @ W1 + b1)        # 4  -> 64
    h2    = relu(h @ W2 + b2)          # 64 -> 64
    delta = h2 @ W3 + b3               # 64 -> 1
Output: deltas (N, T).

Strategy (8 NeuronCores, pure data parallel over N):
  * 32768 paths/core; "pair" = 1024 paths = 2 chunks of 512 stacked on
    the 128 partitions (chunk A -> hidden rows 0:64, chunk B -> 64:128)
  * all matmuls are full-width K=128 (or K<=8 for layer 1) with
    block-diagonal weights -> 3 matmuls x 512 columns per pair-step,
    the minimum PE streaming for this net (PE issue is serial on TRN2)
  * delta feedback is direct: layer-3 deltas for 8 pairs accumulate
    into one [16,512] PSUM tile via column-select lhsT, ACT copies
    them to SBUF (bf16, +b3), small SBUF->SBUF DMAs scatter them into
    the delta rows of the NEXT step's x tile; block ordering hides the
    recurrence latency behind the other blocks' compute
  * x tile per step: [128, 4096] bf16; partition group 32g holds rows
    [fA(3), fB(3), dA, dB] for pairs p=4q+g at free cols 512q
  * features pre-packed on host to [T, 4, 6, 4096] so each step's
    feature load is one large contiguous DMA
"""

import sys

for _p in ("/opt/trn_rl_repo",):
    if _p not in sys.path:
        sys.path.insert(0, _p)

import numpy as np
import ml_dtypes

NCORES = 8
N_TOT, T, FDIM = 262144, 60, 3
NC = N_TOT // NCORES          # 32768 paths per core
HID = 64
CH = 512                      # matmul free dim (one PSUM bank of fp32)
NPAIR = 32                    # pairs (1024 paths) per core-step
NBLK = 2                      # layer-3 blocks of 16 pairs
XPRE = 4                      # steps of feature prefetch

# wpack column layout (bf16, 128 partitions)
W1_OFF = 0                    # cols 0:128, rows 32g+{0..7}: L1 lhsT per group
W2_OFF = 128                  # cols 128:256: blockdiag(W2, W2)
W3_OFF = 256                  # cols 256:768: 16 x [128,32] L3 select lhsT
WCOLS = 768


def _build_graph(nsteps=T, b3val=0.0):
    import concourse.bacc as bacc
    from concourse import mybir
    from concourse.tile import TileContext

    BF = mybir.dt.bfloat16
    F32 = mybir.dt.float32

    import time as _time

    nc = bacc.Bacc(trn_type="TRN2", name=f"k{int(_time.time())}")

    feats_p = nc.declare_dram_parameter("feats", [nsteps, 4, 6, 8 * CH], BF,
                                        isOutput=False)
    wpack_p = nc.declare_dram_parameter("wpack", [128, WCOLS], BF,
                                        isOutput=False)
    bias_p = nc.declare_dram_parameter("biasp", [128, 2], F32, isOutput=False)
    out_p = nc.declare_dram_parameter("out", [nsteps, 32, NBLK * CH], BF,
                                      isOutput=True)

    with TileContext(nc) as tc:
        with (
            tc.tile_pool(name="consts", bufs=1) as cpool,
            tc.tile_pool(name="xq", bufs=XPRE + 2) as xpool,
            tc.tile_pool(name="hh", bufs=3) as hpool,
            tc.tile_pool(name="dst", bufs=2) as dpool,
            tc.tile_pool(name="ps", bufs=3, space="PSUM") as ppool,
            tc.tile_pool(name="ps3", bufs=2, space="PSUM") as ppool3,
        ):
            wp = cpool.tile([128, WCOLS], BF, tag="wpack")
            bp = cpool.tile([128, 2], F32, tag="biasp")
            nc.sync.dma_start(out=wp[:, :], in_=wpack_p[:, :])
            nc.sync.dma_start(out=bp[:, :], in_=bias_p[:, :])

            # Warm-up: loads the ACT table + lets ACT/DVE observe const DMAs
            warm = cpool.tile([128, 4], F32, tag="warm")
            nc.scalar.activation(
                warm[:, 0:1], bp[:, 0:1],
                mybir.ActivationFunctionType.Relu, bias=0.0, scale=1.0,
            )
            nc.vector.tensor_scalar(
                warm[:, 1:2], bp[:, 1:2], 0.0, None, mybir.AluOpType.add,
            )

            # PE warm-up burst: ~8 us of dense back-to-back matmuls trips
            # the HAM activity monitor to K=8/8 (2.4 GHz) before the main
            # loop; overlaps the constant/feature DMAs.  Without it the
            # clock stays at the cold 1.2 GHz default for the whole kernel
            # (HAM is free-running; kernel start state is luck otherwise).
            dum = cpool.tile([128, 640], BF, tag="dum")
            nc.gpsimd.memset(dum[:, :], 0.0)
            Pw = ppool.tile([128, 2 * CH], F32, tag="pre")
            for _ in range(24):
                nc.tensor.matmul(
                    Pw[:, 0:CH], dum[:, 0:128], dum[:, 128:640],
                    start=True, stop=True,
                )

            def dma_x(t0):
                xt = xpool.tile([128, 8 * CH], BF, tag="x")
                for g in range(4):
                    nc.sync.dma_start(
                        out=xt[32 * g: 32 * g + 6, :], in_=feats_p[t0, g]
                    )
                return xt

            xq = [dma_x(t0) for t0 in range(min(XPRE, nsteps))]

            for t in range(nsteps):
                if t + XPRE < nsteps:
                    xq.append(dma_x(t + XPRE))
                xt = xq[t]
                xnext = xq[t + 1] if t + 1 < nsteps else None
                dstage = dpool.tile([32, NBLK * CH], BF, tag="dstage")

                for b in range(NBLK):
                    acc3 = ppool3.tile([32, CH], F32, tag="l3acc")
                    i3 = 0
                    for qd in (2 * b, 2 * b + 1):
                      for g in range(4):
                        # duo: pairs (q=2qd, g) and (q=2qd+1, g)
                        P = ppool.tile([128, 2 * CH], F32, tag="pre")
                        h = hpool.tile([128, 2 * CH], BF, tag="h")
                        h2 = hpool.tile([128, 2 * CH], BF, tag="h2")

                        # ---- layer 1 (K=8; t=0 uses K=6: no delta) ----
                        K1 = 6 if t == 0 else 8
                        for qq in range(2):
                            q = 2 * qd + qq
                            nc.tensor.matmul(
                                P[:, CH * qq: CH * (qq + 1)],
                                wp[32 * g: 32 * g + K1, W1_OFF: W1_OFF + 128],
                                xt[32 * g: 32 * g + K1,
                                   CH * q: CH * (q + 1)],
                                start=True, stop=True,
                                tile_position=(32 * g, 0),
                            )
                        # ---- act1: h = relu(pre1 + b1) on ACT ----
                        nc.scalar.activation(
                            h[:, :], P[:, :],
                            mybir.ActivationFunctionType.Relu,
                            bias=bp[:, 0:1], scale=1.0,
                        )
                        # ---- layer 2 (K=128 blockdiag W2, in-place) ----
                        for qq in range(2):
                            nc.tensor.matmul(
                                P[:, CH * qq: CH * (qq + 1)],
                                wp[:, W2_OFF: W2_OFF + 128],
                                h[:, CH * qq: CH * (qq + 1)],
                                start=True, stop=True,
                            )
                        # ---- act2: h2 = relu(pre2 + b2) on DVE ----
                        nc.vector.tensor_scalar(
                            h2[:, :], P[:, :],
                            bp[:, 1:2], 0.0,
                            mybir.AluOpType.add, mybir.AluOpType.max,
                        )
                        # ---- layer 3: select-accumulate into acc3 ----
                        for qq in range(2):
                            qloc = 2 * (qd - 2 * b) + qq
                            k = 4 * qloc + g
                            nc.tensor.matmul(
                                acc3[0:32, :],
                                wp[:, W3_OFF + 32 * k: W3_OFF + 32 * (k + 1)],
                                h2[:, CH * qq: CH * (qq + 1)],
                                start=(i3 == 0), stop=(i3 == 15),
                            )
                            i3 += 1
                    # ---- act3: dstage block = acc3 + b3 (bf16) ----
                    nc.scalar.activation(
                        dstage[0:32, CH * b: CH * (b + 1)], acc3[0:32, :],
                        mybir.ActivationFunctionType.Copy,
                        bias=float(b3val), scale=1.0,
                    )
                    # ---- scatter deltas into next step's x tile ----
                    if xnext is not None:
                        for c in range(2):
                            nc.sync.dma_start(
                                out=xnext.rearrange(
                                    "(g r) (q4 m) -> g r q4 m", g=4, q4=8
                                )[:, 6 + c, 4 * b: 4 * b + 4, :],
                                in_=dstage[16 * c: 16 * c + 16,
                                           CH * b: CH * (b + 1)],
                            )
                # ---- deltas out ----
                nc.sync.dma_start(out=out_p[t], in_=dstage[0:32, :])
    return nc


LAST_RESULT = None


def kernel(**inputs):
    return _run(inputs, T)


def _prepare(inputs, nsteps):
    features = np.asarray(inputs["features"], dtype=np.float32)
    W1 = np.asarray(inputs["W1"], dtype=np.float32)
    b1 = np.asarray(inputs["b1"], dtype=np.float32)
    W2 = np.asarray(inputs["W2"], dtype=np.float32)
    b2 = np.asarray(inputs["b2"], dtype=np.float32)
    W3 = np.asarray(inputs["W3"], dtype=np.float32)
    b3 = np.asarray(inputs["b3"], dtype=np.float32)

    nc = _build_graph(nsteps, float(b3[0]))
    nc.finalize()

    bf = ml_dtypes.bfloat16

    # wpack
    wpack = np.zeros((128, WCOLS), np.float32)
    for g in range(4):
        for c in range(2):
            for f in range(FDIM):
                wpack[32 * g + 3 * c + f,
                      W1_OFF + 64 * c: W1_OFF + 64 * (c + 1)] = W1[f]
            wpack[32 * g + 6 + c,
                  W1_OFF + 64 * c: W1_OFF + 64 * (c + 1)] = W1[3]
    for c in range(2):
        wpack[64 * c: 64 * (c + 1),
              W2_OFF + 64 * c: W2_OFF + 64 * (c + 1)] = W2
    for k in range(16):  # k = 4*qloc + g; acc3 rows m = 16c + 4g + qloc
        qloc, g = divmod(k, 4)
        for c in range(2):
            m = 16 * c + 4 * g + qloc
            wpack[64 * c: 64 * (c + 1), W3_OFF + 32 * k + m] = W3[:, 0]
    wpack = wpack.astype(bf)

    biasp = np.zeros((128, 2), np.float32)
    for half in (0, 64):
        biasp[half: half + HID, 0] = b1
        biasp[half: half + HID, 1] = b2

    # features: path = 4096 q + 1024 g + 512 c + j
    # host layout F[t, g, 3c+f, 512 q + j] = features[path, t, f]
    in_maps = []
    for k in range(NCORES):
        sh = features[k * NC: (k + 1) * NC, :nsteps, :]   # (NC, ns, 3)
        v = sh.reshape(8, 4, 2, CH, nsteps, FDIM)          # q g c j t f
        v = v.transpose(4, 1, 2, 5, 0, 3)                  # t g c f q j
        feats = np.ascontiguousarray(v).reshape(
            nsteps, 4, 6, 8 * CH).astype(bf)
        in_maps.append({"feats": feats, "wpack": wpack, "biasp": biasp})

    return nc, in_maps


def _unscramble(o, nsteps):
    # o: (ns, 32, 1024) bf16; rows m = 16 c + 4 g + qloc, cols = 512 b + j
    # path = 4096 (4b + qloc) + 1024 g + 512 c + j
    v = np.asarray(o).astype(np.float32)
    v = v.reshape(nsteps, 2, 4, 4, NBLK, CH)        # t c g qloc b j
    v = v.transpose(4, 3, 2, 1, 5, 0)               # b qloc g c j t
    return v.reshape(NC, nsteps)


def _run(inputs, nsteps, trace=False):
    global LAST_RESULT
    from concourse.bass_utils import run_bass_kernel_spmd

    nc, in_maps = _prepare(inputs, nsteps)
    res = run_bass_kernel_spmd(
        nc, in_maps, core_ids=list(range(NCORES)), trace=trace
    )
    LAST_RESULT = res
    outs = res.results

    full = np.empty((N_TOT, nsteps), np.float32)
    for k in range(NCORES):
        full[k * NC: (k + 1) * NC] = _unscramble(outs[k]["out"], nsteps)
    return full


if __name__ == "__main__":
    import reference

    inputs = reference.setup_inputs()
    out = kernel(**{k: np.asarray(v) for k, v in inputs.items()})
    print("kernel out", out.shape, out.dtype)
